# revision 1
# baseline (speedup 1.0000x reference)
"""Bidirectional DSS/Mamba block on 8 trn2 cores (Bass/Tile).

Sharding: core = (batch b = core//2, d_inner half = core%2). Each core
computes the full in-proj for its batch (x is needed in full for x_proj),
scans its 256 d_inner channels in both directions, and produces a partial
(256-channel) contribution to the output projection; the host sums the two
partials per batch. The only cross-core dependency is the global-gate
pooled vector (1024 floats), AllReduce'd over core pairs.

Layout: channels on partitions, sequence L on the free dim. All weight
transposes/permutations happen on the host. W_in/W_xproj columns are
permuted per core so the core's own d-half always occupies x-tiles 0..1,
keeping the program identical across cores (single SPMD NEFF).

Per state index n (1..16): dA_n = exp(A[:,n] * dt) via one ACT op with a
per-partition scale vector; dBu_n = (dt*x) * broadcast(B_n) where the
broadcast of row B_n across 128 partitions is a DMA with a step-0
partition AP from a DRAM bounce; the recurrence h = dA*h + dBu runs as a
DVE tensor_tensor_scan per 128-channel tile (backward direction = same
scan through negative-stride APs, reusing dBu and, when A_f == A_b, dA).
The readout y = sum_n C_n * h_n accumulates in PSUM via PE identity
matmuls, with the D*x skip folded in as a diag(D) matmul.
"""

import os
import sys

sys.path.insert(0, "/opt/trn_rl_repo")

from contextlib import ExitStack

import ml_dtypes
import numpy as np

import concourse.bass as bass
import concourse.bacc as bacc
import concourse.tile as tile
from concourse import mybir
from concourse.bass_utils import run_bass_kernel_spmd

F32 = mybir.dt.float32
BF16 = mybir.dt.bfloat16
AF = mybir.ActivationFunctionType
OP = mybir.AluOpType

B, L, DM, DS, DI, R = 4, 900, 256, 16, 512, 16
DH = DI // 2          # d_inner channels per core
NDT = DH // 128       # 128-channel tiles per core (2)
FCH = [(0, 512), (512, L - 512)]  # PSUM-bank-aligned L chunks


def _bcast_rows(ap):
    """Partition-broadcast AP: read one row 128 times."""
    return bass.AP(tensor=ap.tensor, offset=ap.offset, ap=[[0, 128]] + ap.ap[1:])


PHASE = int(os.environ.get("K_PHASE", "4"))
SUB = set(os.environ.get("K_SUB", "").split(","))


class _SkipRest(Exception):
    pass


def _build_module(shared_a: bool):
    nc = bacc.Bacc("TRN2", num_devices=8)

    ein = lambda n, s: nc.dram_tensor(n, s, F32, kind="ExternalInput")
    ein_bf = lambda n, s: nc.dram_tensor(n, s, BF16, kind="ExternalInput")
    hsT = ein_bf("hsT", [DM, L])
    WinxT = ein_bf("WinxT", [DM, DI])
    WinzT = ein_bf("WinzT", [DM, DH])
    WxT = ein_bf("WxT", [DI, R + 2 * DS])
    WdtT = ein_bf("WdtT", [R, DH])
    bdt = ein("bdt", [128, NDT])
    Afc = ein("Afc", [128, NDT * DS])      # A_f columns per (dtile, n)
    Abc = ein("Abc", [128, NDT * DS])      # A_b columns
    Ddf = ein_bf("Ddf", [DH, 128])
    Ddb = ein_bf("Ddb", [DH, 128])
    I128 = ein_bf("I128", [128, 128])
    G2T = ein("G2T", [2 * DI, 2 * DH])
    bgate2 = ein("bgate2", [1, 2 * DH])
    WoT = ein_bf("WoT", [2 * DH, DM])
    outp = nc.dram_tensor("outp", [DM, L], F32, kind="ExternalOutput")

    bc_dram = nc.dram_tensor("bc_bounce", [2 * DS, L], BF16, kind="Internal")
    u_cc_in = nc.dram_tensor("u_cc_in", [1, 2 * DH], F32, kind="Internal")
    u_cc_out = nc.dram_tensor("u_cc_out", [1, 2 * DI], F32, kind="Internal")
    g_dram = nc.dram_tensor("g_dram", [1, 2 * DH], F32, kind="Internal")

    with ExitStack() as ctx:
        tc = ctx.enter_context(tile.TileContext(nc))
        wpool = ctx.enter_context(tc.tile_pool(name="weights", bufs=1))
        apool = ctx.enter_context(tc.tile_pool(name="acts", bufs=1))

        def load(name, dram, p, f, dt_=None, eng=None):
            ts = []
            for i in range(0, p, 128):
                pp = min(128, p - i)
                t = wpool.tile([pp, f], dt_ or dram.dtype, tag=f"{name}{i}", name=f"{name}{i}")
                (eng or nc.sync).dma_start(out=t, in_=dram[i : i + pp, :])
                ts.append(t)
            return ts

        # order matters: the in-proj inputs go first on the sync queue;
        # the gate/out weights (needed ~200us later) go on the ACT queue
        hs = load("hs", hsT, DM, L)
        winx = load("winx", WinxT, DM, DI)
        winz = load("winz", WinzT, DM, DH)
        wx = load("wx", WxT, DI, R + 2 * DS, eng=nc.scalar)
        wdt = load("wdt", WdtT, R, DH, eng=nc.scalar)
        bdt_s = load("bdt", bdt, 128, NDT)[0]
        af_s = load("afc", Afc, 128, NDT * DS)[0]
        ab_s = load("abc", Abc, 128, NDT * DS)[0]
        ddf = load("ddf", Ddf, DH, 128, eng=nc.scalar)
        ddb = load("ddb", Ddb, DH, 128, eng=nc.scalar)
        ident = load("ident", I128, 128, 128, eng=nc.scalar)[0]
        wo = load("wo", WoT, 2 * DH, DM, eng=nc.gpsimd)
        g2 = load("g2", G2T, 2 * DI, 2 * DH, eng=nc.gpsimd)
        bgate_r = load("bgate2", bgate2, 1, 2 * DH, eng=nc.gpsimd)

        # ---- in-proj: x (full DI, silu'd; own half = tiles 0..1) + z half ----
        xT = [apool.tile([128, L], BF16, tag=f"xT{i}", name=f"xT{i}") for i in range(4)]
        zg = [apool.tile([128, L], BF16, tag=f"zg{i}", name=f"zg{i}") for i in range(NDT)]
        with tc.tile_pool(name="ps_early", bufs=2, space="PSUM") as ps_early:
            for pc in range(6):
                ps = ps_early.tile([128, L], F32, tag="xz", name="xz")
                for f0, fl in FCH:
                    for kc in range(2):
                        lhsT = (
                            winx[kc][:, pc * 128 : (pc + 1) * 128]
                            if pc < 4
                            else winz[kc][:, (pc - 4) * 128 : (pc - 3) * 128]
                        )
                        nc.tensor.matmul(
                            ps[:, f0 : f0 + fl],
                            lhsT,
                            hs[kc][:, f0 : f0 + fl],
                            start=(kc == 0),
                            stop=(kc == 1),
                        )
                dst = xT[pc] if pc < 4 else zg[pc - 4]
                nc.scalar.activation(dst, ps, AF.Silu)

            if PHASE == 0:
                for dtc in range(NDT):
                    nc.sync.dma_start(
                        out=outp[dtc * 128 : (dtc + 1) * 128, :], in_=xT[dtc]
                    )
            if PHASE >= 1:
                # ---- x_proj -> x_dbl [48, L]; bounce B/C rows to DRAM ----
                xdbl = apool.tile([R + 2 * DS, L], BF16, tag="xdbl", name="xdbl")
                ps = ps_early.tile([R + 2 * DS, L], F32, tag="aux", name="aux")
                for f0, fl in FCH:
                    for kc in range(4):
                        nc.tensor.matmul(
                            ps[:, f0 : f0 + fl],
                            wx[kc],
                            xT[kc][:, f0 : f0 + fl],
                            start=(kc == 0),
                            stop=(kc == 3),
                        )
                nc.scalar.activation(xdbl, ps[0 : R + 2 * DS, :], AF.Copy)
                nc.sync.dma_start(out=bc_dram[:, :], in_=xdbl[R : R + 2 * DS, :])

                # ---- dt = softplus(dt_r @ WdtT + bdt) ----
                dtT = [apool.tile([128, L], BF16, tag=f"dtT{i}", name=f"dtT{i}") for i in range(NDT)]
                for dtc in range(NDT):
                    ps = ps_early.tile([128, L], F32, tag="aux", name="aux")
                    for f0, fl in FCH:
                        nc.tensor.matmul(
                            ps[:, f0 : f0 + fl],
                            wdt[0][:, dtc * 128 : (dtc + 1) * 128],
                            xdbl[0:R, f0 : f0 + fl],
                            start=True,
                            stop=True,
                        )
                    # softplus(v+b) = ln(1 + exp(v+b)) in fp32 (bf16 would
                    # cancel 1+e^v for small dt); downcast only the final dt
                    sp = apool.tile([128, L], F32, tag="sp_tmp", name="sp_tmp")
                    nc.scalar.activation(
                        sp, ps, AF.Exp, bias=bdt_s[:, dtc : dtc + 1]
                    )
                    nc.vector.tensor_scalar_add(sp, sp, 1.0)
                    nc.scalar.activation(dtT[dtc], sp, AF.Ln)

        if PHASE == 1:
            for dtc in range(NDT):
                nc.sync.dma_start(
                    out=outp[dtc * 128 : (dtc + 1) * 128, :], in_=dtT[dtc]
                )
        # w = dt * x_own
        if PHASE >= 1:
            w2 = [apool.tile([128, L], BF16, tag=f"w2{i}", name=f"w2{i}") for i in range(NDT)]
            for dtc in range(NDT):
                nc.vector.tensor_mul(w2[dtc], dtT[dtc], xT[dtc])

        # ---- scan loop over state index n ----
        ypsum = {}
        if PHASE >= 2:
            with tc.tile_pool(name="ps_y", bufs=1, space="PSUM") as ps_y, \
                 tc.tile_pool(name="bcast", bufs=3) as bcast_pool, \
                 tc.tile_pool(name="da", bufs=3) as da_pool, \
                 tc.tile_pool(name="dbu", bufs=3) as dbu_pool, \
                 tc.tile_pool(name="h", bufs=3) as h_pool:
                for dr in range(2):
                    for dtc in range(NDT):
                        yp = ps_y.tile([128, L], F32, tag=f"y{dr}{dtc}", name=f"y{dr}{dtc}")
                        ypsum[(dr, dtc)] = yp
                        dd = (ddf if dr == 0 else ddb)[dtc]
                        for f0, fl in FCH:
                            nc.tensor.matmul(
                                yp[:, f0 : f0 + fl],
                                dd,
                                xT[dtc][:, f0 : f0 + fl],
                                start=True,
                                stop=False,
                                skip_group_check=True,
                            )
                for n in range(DS if PHASE >= 3 else 2):
                    brep = bcast_pool.tile([128, L], BF16, tag="brep", name="brep")
                    crep = bcast_pool.tile([128, L], BF16, tag="crep", name="crep")
                    if "nobcast" in SUB:
                        nc.vector.memset(brep, 0.01)
                        nc.vector.memset(crep, 0.01)
                    else:
                        nc.sync.dma_start(out=brep, in_=_bcast_rows(bc_dram[n : n + 1, :]))
                        nc.sync.dma_start(
                            out=crep, in_=_bcast_rows(bc_dram[DS + n : DS + n + 1, :])
                        )
                    for dtc in range(NDT):
                        col = dtc * DS + n
                        daf = da_pool.tile([128, L], F32, tag=f"daf{dtc}", name=f"daf{dtc}")
                        nc.scalar.activation(
                            daf, dtT[dtc], AF.Exp, scale=af_s[:, col : col + 1]
                        )
                        if shared_a:
                            dab = daf
                        else:
                            dab = da_pool.tile([128, L], F32, tag=f"dab{dtc}", name=f"dab{dtc}")
                            nc.scalar.activation(
                                dab, dtT[dtc], AF.Exp, scale=ab_s[:, col : col + 1]
                            )
                        dbu = dbu_pool.tile([128, L], BF16, tag=f"dbu{dtc}", name=f"dbu{dtc}")
                        nc.vector.tensor_mul(dbu, w2[dtc], brep)
                        for dr in range(2):
                            h = h_pool.tile([128, L], BF16, tag=f"h{dr}{dtc}", name=f"h{dr}{dtc}")
                            if dr == 0:
                                nc.vector.tensor_tensor_scan(
                                    h, daf, dbu, 0.0, OP.mult, OP.add
                                )
                            elif "norev" in SUB:
                                nc.vector.tensor_tensor_scan(
                                    h, dab, dbu, 0.0, OP.mult, OP.add
                                )
                            else:
                                nc.vector.tensor_tensor_scan(
                                    h[:, ::-1], dab[:, ::-1], dbu[:, ::-1],
                                    0.0, OP.mult, OP.add,
                                )
                            nc.vector.tensor_mul(h, h, crep)  # in-place h *= C_n
                            yp = ypsum[(dr, dtc)]
                            for f0, fl in FCH:
                                nc.tensor.matmul(
                                    yp[:, f0 : f0 + fl],
                                    ident,
                                    h[:, f0 : f0 + fl],
                                    start=False,
                                    stop=(n == (DS if PHASE >= 3 else 2) - 1),
                                    skip_group_check=True,
                                )

                # ---- gate: yg = y*zg (+ pooled sum), partial W_global matvec ----
                yg = {}
                m_sb = apool.tile([128, 4], F32, tag="m", name="m")  # cols f0,f1,b0,b1
                for dr in range(2):
                    for dtc in range(NDT):
                        t = apool.tile([128, L], BF16, tag=f"yg{dr}{dtc}", name=f"yg{dr}{dtc}")
                        yg[(dr, dtc)] = t
                        nc.vector.tensor_mul(t, ypsum[(dr, dtc)], zg[dtc])
                        nc.scalar.activation(
                            t, t, AF.Copy,
                            accum_out=m_sb[:, 2 * dr + dtc : 2 * dr + dtc + 1],
                        )

        if PHASE == 2:
            for dtc in range(NDT):
                nc.sync.dma_start(
                    out=outp[dtc * 128 : (dtc + 1) * 128, :], in_=yg[(0, dtc)]
                )
        if PHASE >= 3:
            with tc.tile_pool(name="ps_tail", bufs=1, space="PSUM") as ps_tail:
                # pairwise AllGather of m, then one G2=Wgate@Wglobal matvec
                nc.sync.dma_start(
                    out=bass.AP(tensor=u_cc_in, offset=0,
                                ap=[[1, 128], [128, 4]]),
                    in_=m_sb,
                )
                if PHASE >= 4:
                    nc.gpsimd.collective_compute(
                        "AllGather",
                        OP.bypass,
                        replica_groups=[[0, 1], [2, 3], [4, 5], [6, 7]],
                        ins=[u_cc_in[:, :]],
                        outs=[u_cc_out[:, :]],
                    )
                u2 = apool.tile([128, 8], F32, tag="u2", name="u2")
                if PHASE >= 4:
                    nc.sync.dma_start(
                        out=u2,
                        in_=bass.AP(tensor=u_cc_out, offset=0,
                                    ap=[[1, 128], [128, 8]]),
                    )
                else:
                    nc.vector.memset(u2, 0.0)

                # v as a single [1,512] row: u2 columns are the stationary
                # operand, G2 tiles stream -> 8 wide matmuls, no LDW wall
                vps = ps_tail.tile([1, 2 * DH], F32, tag="vps", name="vps")
                for kc in range(8):
                    nc.tensor.matmul(
                        vps,
                        u2[:, kc : kc + 1],
                        g2[kc],
                        start=(kc == 0),
                        stop=(kc == 7),
                    )
                g_row = apool.tile([1, 2 * DH], F32, tag="grow", name="grow")
                nc.vector.tensor_add(g_row, vps, bgate_r[0])
                nc.scalar.activation(g_row, g_row, AF.Sigmoid)
                nc.sync.dma_start(out=g_dram[:, :], in_=g_row)
                g_sb = apool.tile([128, 4], F32, tag="g", name="g")
                nc.sync.dma_start(
                    out=g_sb,
                    in_=bass.AP(tensor=g_dram, offset=0,
                                ap=[[1, 128], [128, 4]]),
                )

                for dr in range(2):
                    for dtc in range(NDT):
                        c = 2 * dr + dtc
                        nc.vector.tensor_scalar_mul(
                            yg[(dr, dtc)], yg[(dr, dtc)], g_sb[:, c : c + 1]
                        )
                out_sb = [apool.tile([128, L], F32, tag=f"o{i}", name=f"o{i}")
                          for i in range(2)]
                for pc in range(2):
                    ops_ = ps_tail.tile([128, L], F32, tag="ops", name="ops")
                    for f0, fl in FCH:
                        for kc in range(4):
                            nc.tensor.matmul(
                                ops_[:, f0 : f0 + fl],
                                wo[kc][:, pc * 128 : (pc + 1) * 128],
                                yg[(kc // 2, kc % 2)][:, f0 : f0 + fl],
                                start=(kc == 0),
                                stop=(kc == 3),
                            )
                    nc.scalar.activation(out_sb[pc], ops_, AF.Copy)
                    nc.sync.dma_start(
                        out=outp[pc * 128 : (pc + 1) * 128, :], in_=out_sb[pc]
                    )

    nc.finalize()
    return nc


_NC_CACHE = {}


def _get_module(shared_a: bool):
    if shared_a not in _NC_CACHE:
        _NC_CACHE[shared_a] = _build_module(shared_a)
    return _NC_CACHE[shared_a]


def kernel(**inputs):
    inp = {k: np.asarray(v, dtype=np.float32) for k, v in inputs.items()}
    hs = inp["hidden_states"]
    W_in, W_x, W_dt = inp["W_in"], inp["W_xproj"], inp["W_dt"]
    b_dt = inp["b_dt"]
    A_f = -np.exp(inp["A_log_f"])      # (512, 16)
    A_b = -np.exp(inp["A_log_b"])
    D_f, D_b = inp["D_f"], inp["D_b"]
    W_g, b_g = inp["W_global"], inp["b_global"]
    W_gate, b_gate = inp["W_gate"], inp["b_gate"]
    W_out = inp["W_out"]

    shared_a = bool(np.array_equal(A_f, A_b))
    I = np.eye(128, dtype=np.float32)
    in_maps = []
    for core in range(8):
        b, h = core // 2, core % 2
        o = h * DH                      # own-half offset in d_inner
        perm = np.r_[o : o + DH, (DH - o) % DI : (DH - o) % DI + DH]  # own first
        ownc = np.r_[o : o + DH, DI + o : DI + o + DH]  # own rows of 2*DI concat
        ccorder = np.r_[0:DH, DI : DI + DH, DH:DI, DI + DH : 2 * DI]

        def acol(A):
            # [128, NDT*DS]: col (dtc*DS + n) = A[own dtile dtc, n]
            a = A[o : o + DH].reshape(NDT, 128, DS)
            return np.ascontiguousarray(a.transpose(1, 0, 2).reshape(128, NDT * DS))

        bf = ml_dtypes.bfloat16
        m = {
            "hsT": np.ascontiguousarray(hs[b].T).astype(bf),
            "WinxT": np.ascontiguousarray(W_in[:DI][perm].T).astype(bf),
            "WinzT": np.ascontiguousarray(W_in[DI + o : DI + o + DH].T).astype(bf),
            "WxT": np.ascontiguousarray(W_x[:, perm].T).astype(bf),
            "WdtT": np.ascontiguousarray(W_dt[o : o + DH].T).astype(bf),
            "bdt": np.ascontiguousarray(b_dt[o : o + DH].reshape(NDT, 128).T),
            "Afc": acol(A_f),
            "Abc": acol(A_b),
            "Ddf": _diag_stack(D_f[o : o + DH]).astype(bf),
            "Ddb": _diag_stack(D_b[o : o + DH]).astype(bf),
            "I128": I.astype(bf),
            "G2T": np.ascontiguousarray(
                (W_gate[ownc] @ W_g[:, ccorder] / np.float32(L)).T
            ),
            "bgate2": np.ascontiguousarray(
                (b_gate[ownc] + W_gate[ownc] @ b_g).reshape(1, 512)
            ),
            "WoT": np.ascontiguousarray(W_out[:, ownc].T).astype(bf),
        }
        in_maps.append(m)

    nc = _get_module(shared_a)
    res = run_bass_kernel_spmd(nc, in_maps, core_ids=list(range(8)))
    outs = res.results
    out = np.zeros((B, L, DM), dtype=np.float32)
    for b in range(B):
        part = outs[2 * b]["outp"] + outs[2 * b + 1]["outp"]
        out[b] = part.T
    return out


def _diag_stack(d):
    out = np.zeros((DH, 128), dtype=np.float32)
    for t in range(NDT):
        out[t * 128 : (t + 1) * 128, :] = np.diag(d[t * 128 : (t + 1) * 128])
    return out

